# revision 31
# baseline (speedup 1.0000x reference)
"""Multi-head causal self-attention (GPT-2 style block) for 8 Trainium2 cores.

Strategy: pure data-parallel over batch (B=8 -> one batch element per core,
no collectives). Per core, everything is computed in "transposed" layouts so
no on-device transposes are needed:

  Q^T,K^T tiles [128, 1024] (f32r) directly from W_attn-as-lhsT matmuls
  V natural [q, 768] (bf16 V' with a ones column per head for softmax sums)
  S^T[kk, q] = K^T-slice^T @ Q^T-slice (f32r); the causal mask is applied by
      one extra accumulated bf16 matmul whose product is -1e30*max(0, kk-c)
  P^T = exp(S^T / 8)  (ScalarE, bf16 out)
  PV: out^T[d(+ones row), q] accumulated over key tiles (bf16)
  softmax normalization: reciprocal of the ones-row, PE K=1 ones-broadcast,
      fused into the PSUM->SBUF copyback multiply
  out = valsT^T @ W_proj + b  (f32r)

Inputs are DMA'd in per-k-tile pieces so compute starts while loads stream;
tiny bf16 "lane cover" matmuls absorb the per-piece DMA-sem waits (f32r
self-loading matmuls can carry only one sync wait).

Outputs (per core): out [1024,768], K^T [768,1024], V [1024,768]; the host
reassembles (out, K, V) in the reference's shapes.
"""
import numpy as np
import ml_dtypes
import concourse.bacc as bacc
import concourse.mybir as mybir
import concourse.tile as tile
from concourse.bass_utils import run_bass_kernel_spmd

F32 = mybir.dt.float32
F32R = mybir.dt.float32r
BF16 = mybir.dt.bfloat16
AF = mybir.ActivationFunctionType

B, T, D = 8, 1024, 768
H, DH = 12, 64
NCORES = 8

# --- score-tile packing plan (per head) ---
SEG = {0: [512, 512], 1: [512, 384], 2: [512, 256], 3: [384, 256],
       4: [512], 5: [384], 6: [256], 7: [128]}
# kb3 sits at offset 128 so both its pieces stay inside one PSUM bank each
LOC = {0: (0, 0), 1: (1, 0), 2: (2, 0), 3: (3, 128), 4: (4, 0),
       5: (4, 512), 6: (3, 768), 7: (2, 768)}
T2K = {0: [0], 1: [1], 2: [2, 7], 3: [3, 6], 4: [4, 5]}
# per score tile: (exp start col, exp width)
EXPR = [(0, 1024), (0, 896), (0, 896), (128, 896), (0, 896)]

# xw input layout (f32r): [ xT (6*1024) | Wqk (6*1536) | Wv (6*768) ]
XT_OFF = 0
WQK_OFF = 6144
WV_OFF = 15360
XW_COLS = 19968
BCON_COLS = 1548

_CACHE = {}


def _build():
    nc = bacc.Bacc(target_bir_lowering=False)
    xw_d = nc.dram_tensor("xw", [128, XW_COLS], F32R, kind="ExternalInput")
    wp_d = nc.dram_tensor("wpi", [128, 4608], F32R, kind="ExternalInput")
    bcon_d = nc.dram_tensor("bcon", [128, BCON_COLS], F32, kind="ExternalInput")
    cbf_d = nc.dram_tensor("cbf", [128, 256], BF16, kind="ExternalInput")
    out_d = nc.dram_tensor("out", [T, D], F32, kind="ExternalOutput")
    kt_d = nc.dram_tensor("kt", [D, T], F32, kind="ExternalOutput")
    v_d = nc.dram_tensor("v", [T, D], F32, kind="ExternalOutput")

    with tile.TileContext(nc) as tc:
        with tc.tile_pool(name="keep", bufs=1) as keep, \
             tc.tile_pool(name="stg", bufs=3) as stg:
            qk_t = [keep.tile([128, 1024], F32R, tag=f"qk{i}", name=f"qk{i}")
                    for i in range(12)]
            vp_t = [keep.tile([128, 780], BF16, tag=f"vp{q}", name=f"vp{q}")
                    for q in range(8)]
            vt_t = [keep.tile([128, 1024], F32R, tag=f"vt{t}", name=f"vt{t}")
                    for t in range(6)]
            wp_sb = keep.tile([128, 4608], F32R, tag="wp")
            bcon_sb = keep.tile([128, BCON_COLS], F32, tag="bcon")
            cbf_sb = keep.tile([128, 256], BF16, tag="cbf")
            ones_sb = keep.tile([1, 64], F32R, tag="ones")

            nc.gpsimd.dma_start(bcon_sb[:], bcon_d[:])
            nc.gpsimd.dma_start(cbf_sb[:], cbf_d[:])
            bqk = bcon_sb[:, 0:12]
            bvb = bcon_sb[:, 12:780]
            bpb = bcon_sb[:, 780:1548]
            nc.vector.tensor_scalar(ones_sb[:], bcon_sb[0:1, 0:64], 0.0, 1.0,
                                    mybir.AluOpType.mult, mybir.AluOpType.add)
            L_sl = cbf_sb[:, 0:128]
            R_sl = cbf_sb[:, 128:256]

            with tc.tile_pool(name="xa", bufs=1) as xa, \
                 tc.tile_pool(name="psA", bufs=1, space="PSUM") as psA:
                xw_sb = xa.tile([128, XW_COLS], F32R, tag="xw")

                def ld(lo, hi):
                    nc.sync.dma_start(xw_sb[:, lo:hi], xw_d[:, lo:hi])

                # pair-0 path first: xt_t (in 512-col halves) + its (K6|Q0)
                # 256-col block per k tile
                for t in range(6):
                    ld(XT_OFF + t * 1024, XT_OFF + t * 1024 + 512)
                    base = WQK_OFF + t * 1536
                    ld(base, base + 256)
                    ld(XT_OFF + t * 1024 + 512, XT_OFF + (t + 1) * 1024)
                # pair-1 blocks, then V weights, then the remaining pairs
                for t in range(6):
                    base = WQK_OFF + t * 1536 + 256
                    ld(base, base + 256)
                for t in range(6):
                    ld(WV_OFF + t * 768, WV_OFF + (t + 1) * 768)
                for p in range(2, 6):
                    for t in range(6):
                        base = WQK_OFF + t * 1536 + p * 256
                        ld(base, base + 256)
                xt = xw_sb[:, XT_OFF:XT_OFF + 6144]
                wqk = xw_sb[:, WQK_OFF:WQK_OFF + 9216]
                wv = xw_sb[:, WV_OFF:WV_OFF + 4608]

                junk = psA.tile([2, 512], F32, tag="junk", name="junk")

                covered = set()

                def cover(key, ap):
                    # bf16 matmul (LDW+MM) whose moving operand reads one
                    # element of the freshly-DMA'd piece: the MM carries the
                    # (2-slot) DMAHW wait; later f32r matmuls on PE find the
                    # lane already covered.
                    if key in covered:
                        return
                    covered.add(key)
                    nc.tensor.matmul(junk[0:1, 0:2],
                                     cbf_sb[0:1, 0:1],
                                     ap[0:1, 0:1].bitcast(BF16),
                                     start=True, stop=True,
                                     skip_group_check=True)

                def qk_tile(i):
                    p_blk = i - 6 if i >= 6 else i   # pair index
                    k_half = 0 if i >= 6 else 128
                    ps = psA.tile([128, 1024], F32, tag="ps_a",
                                  name=f"psqk{i}", bufs=3)
                    for t in range(6):
                        blk = t * 1536 + p_blk * 256
                        cover(("wqk", t, p_blk), wqk[:, blk: blk + 1])
                        lhs = wqk[:, blk + k_half: blk + k_half + 128]
                        for c in range(2):
                            cover(("xt", t, c),
                                  xt[:, t * 1024 + c * 512: t * 1024 + c * 512 + 1])
                            nc.tensor.matmul(
                                ps[:, c * 512:(c + 1) * 512], lhs,
                                xt[:, t * 1024 + c * 512: t * 1024 + (c + 1) * 512],
                                start=(t == 0), stop=(t == 5))
                    # copyback + bias on ScalarE (frees DVE)
                    nc.scalar.activation(qk_t[i][:], ps[:], AF.Identity,
                                         bias=bqk[:, i:i + 1])
                    if i >= 6:
                        j = i - 6
                        nc.gpsimd.dma_start(kt_d[j * 128:(j + 1) * 128, :],
                                            qk_t[i][:].bitcast(F32))

                def v_tile(q):
                    ps = psA.tile([128, 1024], F32, tag="ps_a",
                                  name=f"psv{q}", bufs=3)
                    for t in range(6):
                        cover(("wv", t), wv[:, t * 768: t * 768 + 1])
                        cover(("xt", t, q // 4),
                              xt[:, t * 1024 + (q // 4) * 512:
                                 t * 1024 + (q // 4) * 512 + 1])
                        lhs = xt[:, t * 1024 + q * 128: t * 1024 + (q + 1) * 128]
                        nc.tensor.matmul(ps[:, 0:512], lhs,
                                         wv[:, t * 768: t * 768 + 512],
                                         start=(t == 0), stop=(t == 5))
                        nc.tensor.matmul(ps[:, 512:768], lhs,
                                         wv[:, t * 768 + 512: t * 768 + 768],
                                         start=(t == 0), stop=(t == 5))
                    vst = stg.tile([128, 768], F32, tag="vst")
                    nc.vector.tensor_add(vst[:], ps[:, 0:768], bvb)
                    nc.gpsimd.dma_start(v_d[q * 128:(q + 1) * 128, :], vst[:])
                    vv = vp_t[q][:].rearrange("p (h d) -> p h d", d=65)
                    nc.vector.tensor_add(
                        vv[:, :, 0:64],
                        ps[:, 0:768].rearrange("p (h d) -> p h d", d=64),
                        bvb.rearrange("p (h d) -> p h d", d=64))
                    nc.vector.tensor_scalar(
                        vv[:, :, 64:65],
                        bcon_sb[:, 0:12].rearrange("p (h o) -> p h o", o=1),
                        0.0, 1.0, mybir.AluOpType.mult, mybir.AluOpType.add)

                # first two pairs' K/Q tiles, then V (wv lands meanwhile)
                qk_tile(6)
                qk_tile(0)
                qk_tile(7)
                qk_tile(1)
                for q in range(8):
                    v_tile(q)
                for p in range(2, 6):
                    qk_tile(6 + p)
                    qk_tile(p)

            # load W_proj now that xa's space is free; DVE-copy so the
            # projection matmuls wait on a single (DVE) sem
            with tc.tile_pool(name="wpl", bufs=1) as wpl:
                wpl_t = wpl.tile([128, 4608], F32R, tag="wpl")
                nc.gpsimd.dma_start(wpl_t[:], wp_d[:])
                nc.vector.tensor_copy(wp_sb[:], wpl_t[:])

                # ---- attention, head pairs ----
                with tc.tile_pool(name="ptp", bufs=21) as ptp, \
                     tc.tile_pool(name="rrp", bufs=3) as rrp, \
                     tc.tile_pool(name="rbs", bufs=3) as rbs, \
                     tc.tile_pool(name="psB", bufs=1, space="PSUM") as psB:
                    for p in range(6):
                        heads = (2 * p, 2 * p + 1)
                        pts = {h: [] for h in heads}
                        for t in range(5):
                            for hi, h in enumerate(heads):
                                qt = qk_t[p]
                                kt = qk_t[6 + p]
                                r0 = (h % 2) * 64
                                sc = psB.tile([128, 1024], F32,
                                              tag=("scA" if hi == 0 else "scB"))
                                for kb in T2K[t]:
                                    off = LOC[kb][1]
                                    nc.tensor.matmul(sc[:, off:off + 128],
                                                     L_sl, R_sl,
                                                     start=True, stop=False,
                                                     skip_group_check=True)
                                    o2 = off
                                    segs = SEG[kb]
                                    for si, w in enumerate(segs):
                                        nc.tensor.matmul(
                                            sc[:, o2:o2 + w],
                                            kt[r0:r0 + 64,
                                               kb * 128:(kb + 1) * 128],
                                            qt[r0:r0 + 64,
                                               kb * 128 + (o2 - off):
                                               kb * 128 + (o2 - off) + w],
                                            start=(o2 != off),
                                            stop=(si == len(segs) - 1),
                                            skip_group_check=True)
                                        o2 += w
                                ptile = ptp.tile([128, 1024], BF16, tag="pt")
                                e0, ew = EXPR[t]
                                nc.scalar.activation(ptile[:, e0:e0 + ew],
                                                     sc[:, e0:e0 + ew], AF.Exp,
                                                     scale=0.125)
                                pts[h].append(ptile)
                        for h in heads:
                            rr = rrp.tile([1, 1024], F32R, tag="rr")
                            for qc in range(2):
                                pvt = psB.tile([65, 512], F32, tag="pv", bufs=3)
                                kbs = list(range(4 * qc + 4))
                                for ki, kb in enumerate(kbs):
                                    q_lo = max(qc * 512, kb * 128)
                                    w = qc * 512 + 512 - q_lo
                                    ti, toff = LOC[kb]
                                    rhs = pts[h][ti][:,
                                                     toff + (q_lo - kb * 128):
                                                     toff + (q_lo - kb * 128) + w]
                                    nc.tensor.matmul(
                                        pvt[0:65,
                                            q_lo - qc * 512:
                                            q_lo - qc * 512 + w],
                                        vp_t[kb][:, h * 65:(h + 1) * 65], rhs,
                                        start=(ki == 0),
                                        stop=(ki == len(kbs) - 1),
                                        skip_group_check=True)
                                with nc.allow_low_precision(reason="f32r recip"):
                                    nc.vector.reciprocal(
                                        rr[:, qc * 512:(qc + 1) * 512],
                                        pvt[64:65, :])
                                rbp = psB.tile([64, 512], F32, tag="rb", bufs=1)
                                nc.tensor.matmul(rbp[:], ones_sb[:],
                                                 rr[:, qc * 512:(qc + 1) * 512],
                                                 start=True, stop=True)
                                rbt = rbs.tile([64, 512], F32, tag="rbs")
                                nc.vector.tensor_copy(rbt[:], rbp[:])
                                vt_sl = vt_t[p][(h % 2) * 64:(h % 2) * 64 + 64,
                                                qc * 512:(qc + 1) * 512]
                                nc.vector.tensor_mul(vt_sl, pvt[0:64, :], rbt[:])

            # ---- output projection ----
            with tc.tile_pool(name="psC", bufs=3, space="PSUM") as psC, \
                 tc.tile_pool(name="ostp", bufs=3) as ostp:
                for m in range(8):
                    ps = psC.tile([128, 768], F32, tag="pr")
                    for t in range(6):
                        lhs = vt_t[t][:, m * 128:(m + 1) * 128]
                        nc.tensor.matmul(ps[:, 0:512], lhs,
                                         wp_sb[:, t * 768: t * 768 + 512],
                                         start=(t == 0), stop=(t == 5))
                        nc.tensor.matmul(ps[:, 512:768], lhs,
                                         wp_sb[:, t * 768 + 512: t * 768 + 768],
                                         start=(t == 0), stop=(t == 5))
                    ost = ostp.tile([128, 768], F32, tag="ost")
                    nc.vector.tensor_add(ost[:], ps[:], bpb)
                    nc.gpsimd.dma_start(out_d[m * 128:(m + 1) * 128, :], ost[:])

    nc.compile()
    return nc


def _prep_shared(W_attn, b_attn, W_proj, b_proj):
    W_attn = np.asarray(W_attn, np.float32)
    W_proj = np.asarray(W_proj, np.float32)
    b_attn = np.asarray(b_attn, np.float32)
    b_proj = np.asarray(b_proj, np.float32)
    # per k-tile, columns grouped by head pair: [K(6+p) 128 | Q(p) 128] x 6
    wqk_n = W_attn[:, :1536].reshape(6, 128, 12, 128)  # [kt, p, mtile, col]
    blocks = []
    for pair in range(6):
        blocks.append(wqk_n[:, :, 6 + pair, :])   # K m-tile
        blocks.append(wqk_n[:, :, pair, :])       # Q m-tile
    wqk = np.stack(blocks, axis=2)  # [kt, p, 12, 128]
    wqk = wqk.transpose(1, 0, 2, 3).reshape(128, 9216)
    wv = W_attn[:, 1536:].reshape(6, 128, 768).transpose(1, 0, 2).reshape(128, 4608)
    wp = np.ascontiguousarray(
        W_proj.reshape(6, 128, 768).transpose(1, 0, 2).reshape(128, 4608))
    bcon = np.empty((128, BCON_COLS), np.float32)
    bcon[:, 0:12] = b_attn[:1536].reshape(12, 128).T
    bcon[:, 12:780] = np.broadcast_to(b_attn[1536:], (128, 768))
    bcon[:, 780:1548] = np.broadcast_to(b_proj, (128, 768))
    jj = np.arange(128)
    Lm = np.where(jj[None, :] > jj[:, None], np.float32(-1e30), 0.0)
    Rm = np.where(jj[None, :] <= jj[:, None], 1.0, 0.0)
    cbf = np.concatenate([Lm, Rm], axis=1).astype(ml_dtypes.bfloat16)
    return wv, wqk, wp, bcon, cbf


def kernel(hidden_states, W_attn, b_attn, W_proj, b_proj):
    hidden_states = np.asarray(hidden_states, np.float32)
    wv, wqk, wp, bcon, cbf = _prep_shared(W_attn, b_attn, W_proj, b_proj)

    if "nc" not in _CACHE:
        _CACHE["nc"] = _build()
    nc = _CACHE["nc"]

    in_maps = []
    for b in range(NCORES):
        xt = np.ascontiguousarray(hidden_states[b].T)  # [768, 1024]
        xt = xt.reshape(6, 128, 1024).transpose(1, 0, 2).reshape(128, 6144)
        xw = np.concatenate([xt, wqk, wv], axis=1)
        assert xw.shape == (128, XW_COLS)
        in_maps.append({"xw": xw, "wpi": wp, "bcon": bcon, "cbf": cbf})

    res = run_bass_kernel_spmd(nc, in_maps, core_ids=list(range(NCORES)))

    out = np.empty((B, T, D), np.float32)
    K = np.empty((B, H, T, DH), np.float32)
    V = np.empty((B, H, T, DH), np.float32)
    for b in range(NCORES):
        r = res.results[b]
        out[b] = r["out"]
        K[b] = r["kt"].reshape(H, DH, T).transpose(0, 2, 1)
        V[b] = r["v"].reshape(T, H, DH).transpose(1, 0, 2)
    return out, K, V


# revision 32
# speedup vs baseline: 1.0038x; 1.0038x over previous
"""Multi-head causal self-attention (GPT-2 style block) for 8 Trainium2 cores.

Strategy: pure data-parallel over batch (B=8 -> one batch element per core,
no collectives). Per core, everything is computed in "transposed" layouts so
no on-device transposes are needed:

  Q^T,K^T tiles [128, 1024] (f32r) directly from W_attn-as-lhsT matmuls
  V natural [q, 768] (bf16 V' with a ones column per head for softmax sums)
  S^T[kk, q] = K^T-slice^T @ Q^T-slice (f32r); the causal mask is applied by
      one extra accumulated bf16 matmul whose product is -1e30*max(0, kk-c)
  P^T = exp(S^T / 8)  (ScalarE, bf16 out)
  PV: out^T[d(+ones row), q] accumulated over key tiles (bf16)
  softmax normalization: reciprocal of the ones-row, PE K=1 ones-broadcast,
      fused into the PSUM->SBUF copyback multiply
  out = valsT^T @ W_proj + b  (f32r)

Inputs are DMA'd in per-k-tile pieces so compute starts while loads stream;
tiny bf16 "lane cover" matmuls absorb the per-piece DMA-sem waits (f32r
self-loading matmuls can carry only one sync wait).

Outputs (per core): out [1024,768], K^T [768,1024], V [1024,768]; the host
reassembles (out, K, V) in the reference's shapes.
"""
import numpy as np
import ml_dtypes
import concourse.bacc as bacc
import concourse.mybir as mybir
import concourse.tile as tile
from concourse.bass_utils import run_bass_kernel_spmd

F32 = mybir.dt.float32
F32R = mybir.dt.float32r
BF16 = mybir.dt.bfloat16
AF = mybir.ActivationFunctionType

B, T, D = 8, 1024, 768
H, DH = 12, 64
NCORES = 8

# --- score-tile packing plan (per head) ---
SEG = {0: [512, 512], 1: [512, 384], 2: [512, 256], 3: [384, 256],
       4: [512], 5: [384], 6: [256], 7: [128]}
# kb3 sits at offset 128 so both its pieces stay inside one PSUM bank each
LOC = {0: (0, 0), 1: (1, 0), 2: (2, 0), 3: (3, 128), 4: (4, 0),
       5: (4, 512), 6: (3, 768), 7: (2, 768)}
T2K = {0: [0], 1: [1], 2: [2, 7], 3: [3, 6], 4: [4, 5]}
# per score tile: (exp start col, exp width)
EXPR = [(0, 1024), (0, 896), (0, 896), (128, 896), (0, 896)]

# xw input layout (f32r): [ xT (6*1024) | Wqk (6*1536) | Wv (6*768) ]
XT_OFF = 0
WQK_OFF = 6144
WV_OFF = 15360
XW_COLS = 19968
BCON_COLS = 1548

_CACHE = {}


def _build():
    nc = bacc.Bacc(target_bir_lowering=False)
    xw_d = nc.dram_tensor("xw", [128, XW_COLS], F32R, kind="ExternalInput")
    wp_d = nc.dram_tensor("wpi", [128, 4608], F32R, kind="ExternalInput")
    bcon_d = nc.dram_tensor("bcon", [128, BCON_COLS], F32, kind="ExternalInput")
    cbf_d = nc.dram_tensor("cbf", [128, 256], BF16, kind="ExternalInput")
    out_d = nc.dram_tensor("out", [T, D], F32, kind="ExternalOutput")
    kt_d = nc.dram_tensor("kt", [D, T], F32, kind="ExternalOutput")
    v_d = nc.dram_tensor("v", [T, D], F32, kind="ExternalOutput")

    with tile.TileContext(nc) as tc:
        with tc.tile_pool(name="keep", bufs=1) as keep, \
             tc.tile_pool(name="stg", bufs=3) as stg:
            qk_t = [keep.tile([128, 1024], F32R, tag=f"qk{i}", name=f"qk{i}")
                    for i in range(12)]
            vp_t = [keep.tile([128, 780], BF16, tag=f"vp{q}", name=f"vp{q}")
                    for q in range(8)]
            vt_t = [keep.tile([128, 1024], F32R, tag=f"vt{t}", name=f"vt{t}")
                    for t in range(6)]
            wp_sb = keep.tile([128, 4608], F32R, tag="wp")
            bcon_sb = keep.tile([128, BCON_COLS], F32, tag="bcon")
            cbf_sb = keep.tile([128, 256], BF16, tag="cbf")
            ones_sb = keep.tile([1, 64], F32R, tag="ones")

            nc.gpsimd.dma_start(bcon_sb[:], bcon_d[:])
            nc.gpsimd.dma_start(cbf_sb[:], cbf_d[:])
            bqk = bcon_sb[:, 0:12]
            bvb = bcon_sb[:, 12:780]
            bpb = bcon_sb[:, 780:1548]
            nc.vector.tensor_scalar(ones_sb[:], bcon_sb[0:1, 0:64], 0.0, 1.0,
                                    mybir.AluOpType.mult, mybir.AluOpType.add)
            L_sl = cbf_sb[:, 0:128]
            R_sl = cbf_sb[:, 128:256]

            with tc.tile_pool(name="xa", bufs=1) as xa, \
                 tc.tile_pool(name="psA", bufs=1, space="PSUM") as psA:
                xw_sb = xa.tile([128, XW_COLS], F32R, tag="xw")

                def ld(lo, hi):
                    nc.sync.dma_start(xw_sb[:, lo:hi], xw_d[:, lo:hi])

                # pair-0 path first: xt_t (in 512-col halves) + its (K6|Q0)
                # 256-col block per k tile
                for t in range(6):
                    ld(XT_OFF + t * 1024, XT_OFF + t * 1024 + 512)
                    base = WQK_OFF + t * 1536
                    ld(base, base + 256)
                    ld(XT_OFF + t * 1024 + 512, XT_OFF + (t + 1) * 1024)
                # pair-1 blocks, then V weights, then the remaining pairs
                for t in range(6):
                    base = WQK_OFF + t * 1536 + 256
                    ld(base, base + 256)
                for t in range(6):
                    ld(WV_OFF + t * 768, WV_OFF + (t + 1) * 768)
                for p in range(2, 6):
                    for t in range(6):
                        base = WQK_OFF + t * 1536 + p * 256
                        ld(base, base + 256)
                xt = xw_sb[:, XT_OFF:XT_OFF + 6144]
                wqk = xw_sb[:, WQK_OFF:WQK_OFF + 9216]
                wv = xw_sb[:, WV_OFF:WV_OFF + 4608]

                junk = psA.tile([2, 512], F32, tag="junk", name="junk")

                covered = set()

                def cover(key, ap):
                    # bf16 matmul (LDW+MM) whose moving operand reads one
                    # element of the freshly-DMA'd piece: the MM carries the
                    # (2-slot) DMAHW wait; later f32r matmuls on PE find the
                    # lane already covered.
                    if key in covered:
                        return
                    covered.add(key)
                    nc.tensor.matmul(junk[0:1, 0:2],
                                     cbf_sb[0:1, 0:1],
                                     ap[0:1, 0:1].bitcast(BF16),
                                     start=True, stop=True,
                                     skip_group_check=True)

                def qk_tile(i):
                    p_blk = i - 6 if i >= 6 else i   # pair index
                    k_half = 0 if i >= 6 else 128
                    ps = psA.tile([128, 1024], F32, tag="ps_a",
                                  name=f"psqk{i}", bufs=3)
                    for t in range(6):
                        blk = t * 1536 + p_blk * 256
                        cover(("wqk", t, p_blk), wqk[:, blk: blk + 1])
                        lhs = wqk[:, blk + k_half: blk + k_half + 128]
                        for c in range(2):
                            cover(("xt", t, c),
                                  xt[:, t * 1024 + c * 512: t * 1024 + c * 512 + 1])
                            nc.tensor.matmul(
                                ps[:, c * 512:(c + 1) * 512], lhs,
                                xt[:, t * 1024 + c * 512: t * 1024 + (c + 1) * 512],
                                start=(t == 0), stop=(t == 5))
                    # copyback + bias on ScalarE (frees DVE)
                    nc.scalar.activation(qk_t[i][:], ps[:], AF.Identity,
                                         bias=bqk[:, i:i + 1])
                    if i >= 6:
                        j = i - 6
                        nc.gpsimd.dma_start(kt_d[j * 128:(j + 1) * 128, :],
                                            qk_t[i][:].bitcast(F32))

                def v_tile(q):
                    ps = psA.tile([128, 1024], F32, tag="ps_a",
                                  name=f"psv{q}", bufs=3)
                    for t in range(6):
                        cover(("wv", t), wv[:, t * 768: t * 768 + 1])
                        cover(("xt", t, q // 4),
                              xt[:, t * 1024 + (q // 4) * 512:
                                 t * 1024 + (q // 4) * 512 + 1])
                        lhs = xt[:, t * 1024 + q * 128: t * 1024 + (q + 1) * 128]
                        nc.tensor.matmul(ps[:, 0:512], lhs,
                                         wv[:, t * 768: t * 768 + 512],
                                         start=(t == 0), stop=(t == 5))
                        nc.tensor.matmul(ps[:, 512:768], lhs,
                                         wv[:, t * 768 + 512: t * 768 + 768],
                                         start=(t == 0), stop=(t == 5))
                    vst = stg.tile([128, 768], F32, tag="vst")
                    nc.vector.tensor_add(vst[:], ps[:, 0:768], bvb)
                    nc.gpsimd.dma_start(v_d[q * 128:(q + 1) * 128, :], vst[:])
                    vv = vp_t[q][:].rearrange("p (h d) -> p h d", d=65)
                    nc.vector.tensor_add(
                        vv[:, :, 0:64],
                        ps[:, 0:768].rearrange("p (h d) -> p h d", d=64),
                        bvb.rearrange("p (h d) -> p h d", d=64))
                    nc.vector.tensor_scalar(
                        vv[:, :, 64:65],
                        bcon_sb[:, 0:12].rearrange("p (h o) -> p h o", o=1),
                        0.0, 1.0, mybir.AluOpType.mult, mybir.AluOpType.add)

                # first two pairs' K/Q tiles, then V (wv lands meanwhile)
                qk_tile(6)
                qk_tile(0)
                qk_tile(7)
                qk_tile(1)
                for q in range(8):
                    v_tile(q)
                for p in range(2, 6):
                    qk_tile(6 + p)
                    qk_tile(p)

            # load W_proj now that xa's space is free; DVE-copy so the
            # projection matmuls wait on a single (DVE) sem
            with tc.tile_pool(name="wpl", bufs=1) as wpl:
                wpl_t = wpl.tile([128, 4608], F32R, tag="wpl")
                nc.gpsimd.dma_start(wpl_t[:], wp_d[:])
                nc.vector.tensor_copy(wp_sb[:], wpl_t[:])

                # ---- attention, head pairs ----
                with tc.tile_pool(name="ptp", bufs=21) as ptp, \
                     tc.tile_pool(name="rrp", bufs=3) as rrp, \
                     tc.tile_pool(name="rbs", bufs=3) as rbs, \
                     tc.tile_pool(name="psB", bufs=1, space="PSUM") as psB:
                    for p in range(6):
                        heads = (2 * p, 2 * p + 1)
                        pts = {h: [] for h in heads}
                        for t in range(5):
                            for hi, h in enumerate(heads):
                                qt = qk_t[p]
                                kt = qk_t[6 + p]
                                r0 = (h % 2) * 64
                                sc = psB.tile([128, 1024], F32,
                                              tag=("scA" if hi == 0 else "scB"))
                                for kb in T2K[t]:
                                    off = LOC[kb][1]
                                    nc.tensor.matmul(sc[:, off:off + 128],
                                                     L_sl, R_sl,
                                                     start=True, stop=False,
                                                     skip_group_check=True)
                                    o2 = off
                                    segs = SEG[kb]
                                    for si, w in enumerate(segs):
                                        nc.tensor.matmul(
                                            sc[:, o2:o2 + w],
                                            kt[r0:r0 + 64,
                                               kb * 128:(kb + 1) * 128],
                                            qt[r0:r0 + 64,
                                               kb * 128 + (o2 - off):
                                               kb * 128 + (o2 - off) + w],
                                            start=(o2 != off),
                                            stop=(si == len(segs) - 1),
                                            skip_group_check=True)
                                        o2 += w
                                ptile = ptp.tile([128, 1024], BF16, tag="pt")
                                e0, ew = EXPR[t]
                                nc.scalar.activation(ptile[:, e0:e0 + ew],
                                                     sc[:, e0:e0 + ew], AF.Exp,
                                                     scale=0.125)
                                pts[h].append(ptile)
                        rrs = {}
                        for h in heads:
                            rrs[h] = rrp.tile([1, 1024], F32R, tag="rr",
                                              name=f"rr{h}")
                        for qc, h in [(0, heads[0]), (0, heads[1]),
                                      (1, heads[0]), (1, heads[1])]:
                            rr = rrs[h]
                            if True:
                                pvt = psB.tile([65, 512], F32, tag="pv", bufs=3)
                                kbs = list(range(4 * qc + 4))
                                for ki, kb in enumerate(kbs):
                                    q_lo = max(qc * 512, kb * 128)
                                    w = qc * 512 + 512 - q_lo
                                    ti, toff = LOC[kb]
                                    rhs = pts[h][ti][:,
                                                     toff + (q_lo - kb * 128):
                                                     toff + (q_lo - kb * 128) + w]
                                    nc.tensor.matmul(
                                        pvt[0:65,
                                            q_lo - qc * 512:
                                            q_lo - qc * 512 + w],
                                        vp_t[kb][:, h * 65:(h + 1) * 65], rhs,
                                        start=(ki == 0),
                                        stop=(ki == len(kbs) - 1),
                                        skip_group_check=True)
                                with nc.allow_low_precision(reason="f32r recip"):
                                    nc.vector.reciprocal(
                                        rr[:, qc * 512:(qc + 1) * 512],
                                        pvt[64:65, :])
                                rbp = psB.tile([64, 512], F32, tag="rb", bufs=1)
                                nc.tensor.matmul(rbp[:], ones_sb[:],
                                                 rr[:, qc * 512:(qc + 1) * 512],
                                                 start=True, stop=True)
                                rbt = rbs.tile([64, 512], F32, tag="rbs")
                                nc.vector.tensor_copy(rbt[:], rbp[:])
                                vt_sl = vt_t[p][(h % 2) * 64:(h % 2) * 64 + 64,
                                                qc * 512:(qc + 1) * 512]
                                nc.vector.tensor_mul(vt_sl, pvt[0:64, :], rbt[:])

            # ---- output projection ----
            with tc.tile_pool(name="psC", bufs=3, space="PSUM") as psC, \
                 tc.tile_pool(name="ostp", bufs=3) as ostp:
                for m in range(8):
                    ps = psC.tile([128, 768], F32, tag="pr")
                    for t in range(6):
                        lhs = vt_t[t][:, m * 128:(m + 1) * 128]
                        nc.tensor.matmul(ps[:, 0:512], lhs,
                                         wp_sb[:, t * 768: t * 768 + 512],
                                         start=(t == 0), stop=(t == 5))
                        nc.tensor.matmul(ps[:, 512:768], lhs,
                                         wp_sb[:, t * 768 + 512: t * 768 + 768],
                                         start=(t == 0), stop=(t == 5))
                    ost = ostp.tile([128, 768], F32, tag="ost")
                    nc.vector.tensor_add(ost[:], ps[:], bpb)
                    nc.gpsimd.dma_start(out_d[m * 128:(m + 1) * 128, :], ost[:])

    nc.compile()
    return nc


def _prep_shared(W_attn, b_attn, W_proj, b_proj):
    W_attn = np.asarray(W_attn, np.float32)
    W_proj = np.asarray(W_proj, np.float32)
    b_attn = np.asarray(b_attn, np.float32)
    b_proj = np.asarray(b_proj, np.float32)
    # per k-tile, columns grouped by head pair: [K(6+p) 128 | Q(p) 128] x 6
    wqk_n = W_attn[:, :1536].reshape(6, 128, 12, 128)  # [kt, p, mtile, col]
    blocks = []
    for pair in range(6):
        blocks.append(wqk_n[:, :, 6 + pair, :])   # K m-tile
        blocks.append(wqk_n[:, :, pair, :])       # Q m-tile
    wqk = np.stack(blocks, axis=2)  # [kt, p, 12, 128]
    wqk = wqk.transpose(1, 0, 2, 3).reshape(128, 9216)
    wv = W_attn[:, 1536:].reshape(6, 128, 768).transpose(1, 0, 2).reshape(128, 4608)
    wp = np.ascontiguousarray(
        W_proj.reshape(6, 128, 768).transpose(1, 0, 2).reshape(128, 4608))
    bcon = np.empty((128, BCON_COLS), np.float32)
    bcon[:, 0:12] = b_attn[:1536].reshape(12, 128).T
    bcon[:, 12:780] = np.broadcast_to(b_attn[1536:], (128, 768))
    bcon[:, 780:1548] = np.broadcast_to(b_proj, (128, 768))
    jj = np.arange(128)
    Lm = np.where(jj[None, :] > jj[:, None], np.float32(-1e30), 0.0)
    Rm = np.where(jj[None, :] <= jj[:, None], 1.0, 0.0)
    cbf = np.concatenate([Lm, Rm], axis=1).astype(ml_dtypes.bfloat16)
    return wv, wqk, wp, bcon, cbf


def kernel(hidden_states, W_attn, b_attn, W_proj, b_proj):
    hidden_states = np.asarray(hidden_states, np.float32)
    wv, wqk, wp, bcon, cbf = _prep_shared(W_attn, b_attn, W_proj, b_proj)

    if "nc" not in _CACHE:
        _CACHE["nc"] = _build()
    nc = _CACHE["nc"]

    in_maps = []
    for b in range(NCORES):
        xt = np.ascontiguousarray(hidden_states[b].T)  # [768, 1024]
        xt = xt.reshape(6, 128, 1024).transpose(1, 0, 2).reshape(128, 6144)
        xw = np.concatenate([xt, wqk, wv], axis=1)
        assert xw.shape == (128, XW_COLS)
        in_maps.append({"xw": xw, "wpi": wp, "bcon": bcon, "cbf": cbf})

    res = run_bass_kernel_spmd(nc, in_maps, core_ids=list(range(NCORES)))

    out = np.empty((B, T, D), np.float32)
    K = np.empty((B, H, T, DH), np.float32)
    V = np.empty((B, H, T, DH), np.float32)
    for b in range(NCORES):
        r = res.results[b]
        out[b] = r["out"]
        K[b] = r["kt"].reshape(H, DH, T).transpose(0, 2, 1)
        V[b] = r["v"].reshape(T, H, DH).transpose(1, 0, 2)
    return out, K, V
